# revision 29
# baseline (speedup 1.0000x reference)
# Trainium2 Bass kernel for a pre-norm transformer encoder block.
#
# Sharding: 8 cores = 2 batches x 4 query-blocks of 512 tokens.
# Each core recomputes K/V for its batch (replicated 4x within the batch
# group), which removes every collective; FFN/projection rows are
# disjoint. The per-core query-block offset cannot appear in the
# SPMD-identical program, so the host *rolls* each core's token axis to
# put its query block at columns 0..511 — attention is permutation-
# equivariant over keys when the mask rows are rolled identically.
#
# Host-side prep: all big tensors are cast to bf16; the LayerNorm affine
# (alpha, beta) is folded into the following weight matrices and biases
# (exact algebra), and 1/sqrt(DK) is folded into wq/bq.  On-chip LN
# therefore produces the plain normalized z.  wv's columns are permuted
# so the V projection emits heads in [evens, odds] order (see V65).
#
# The softmax row-sums ride along the attention-output matmul as a ones
# column appended to V (even heads: ones col 64 -> psum row 64 with o at
# rows 0:64; odd heads: ones col 0 -> psum row 63 with o at rows 64:128,
# matching the feature-major o_sb layout).  1/rowsum is computed as
# exp(-ln r) on the scalar engine with a gpsimd partition_broadcast in
# between; LN rstd likewise uses exp(-0.5 ln var).  Ln/Exp share one ACT
# table set with the softmax exp, so no table reload ever happens.

import numpy as np
import ml_dtypes

B, S, D, H, DK, DFF = 2, 2048, 768, 12, 64, 3072
P = 128
TQ = 512               # query rows per core
NCORES = 8
DC = D // P            # 6  d-chunks
FC = DFF // P          # 24 dff-chunks
TC = S // P            # 16 key chunks
NT = S // TQ           # 4  512-wide token slices
LNC = float(D) / (D - 1)   # ddof=1 correction
SCH_A = 128.0 / float(np.log(2.0))   # Schraudolph bf16 scale (folded into wq)
SCH_B = 16256.0 - 7.42               # Schraudolph bf16 bias
WKV_SCALE = 64.0                     # fp8 scale for wk/wv (undone on psum)

_CACHE: dict = {}


def _patch_act_tables():
    """Restrict the ACT table-set chooser to natural_log_exp_and_others
    (contains every activation this kernel uses: exp/ln/relu/copy/
    identity/square), so exactly one ACT_TABLE_LOAD is ever emitted
    instead of thrashing between the exp and ln sets per instruction.
    Set ids are preserved (other sets stay in the dict, just emptied)."""
    import concourse.bacc as bacc_mod
    if getattr(bacc_mod, "_act_tables_patched", False):
        return
    orig = bacc_mod.get_activation_tables

    def _gat(arch):
        t = orig(arch)
        if "natural_log_exp_and_others" in t:
            return {k: (v if k == "natural_log_exp_and_others" else set())
                    for k, v in t.items()}
        return t

    bacc_mod.get_activation_tables = _gat
    bacc_mod._act_tables_patched = True


def _build_module(bench_iters=0, debug=False):
    import concourse.bass as bass
    import concourse.mybir as mybir
    import concourse.tile as tile
    from concourse import bacc
    from contextlib import ExitStack

    _patch_act_tables()

    f32 = mybir.dt.float32
    bf = mybir.dt.bfloat16
    f8 = mybir.dt.float8e4
    i16 = mybir.dt.int16
    A = mybir.AluOpType
    AF = mybir.ActivationFunctionType
    DR = mybir.MatmulPerfMode.DoubleRow

    nc = bacc.Bacc(trn_type="TRN2", num_swdge_queues=4)

    def din(name, shape, dt=bf):
        return nc.dram_tensor(name, shape, dt, kind="ExternalInput").ap()

    hT_d = din("hT", [D, S], f8)    # LN1(x) precomputed on host, fp8
    xres_d = din("xres", [D, TQ])   # residual slice of x
    mT_d = din("maskT", [S, TQ])
    w_d = {n: din(n, [D, D], f8 if n in ("wq", "wk", "wv") else bf)
           for n in ["wq", "wk", "wv", "wo"]}
    w1_d = din("w1", [D, DFF])
    w2_d = din("w2", [DFF, D])
    vec_d = {n: din(n, [P, DC], f32) for n in ["bq", "bk", "bo", "b2"]}
    b1_d = din("b1", [P, FC], f32)
    bvb_d = din("bvb", [P, D])
    outT_d = nc.dram_tensor("outT", [D, TQ], f32, kind="ExternalOutput").ap()
    dbg = {}
    if debug:
        for n, shp in [("d_hT", [P, DC, S]), ("d_K", [P, DC, S]),
                       ("d_Q", [P, DC, TQ]), ("d_V", [P, TC, H, 65]),
                       ("d_osb", [P, DC, TQ]), ("d_rb", [P, TQ])]:
            dbg[n] = nc.dram_tensor(n, shp, mybir.dt.float32,
                                    kind="ExternalOutput").ap()
        dbg["d_x2"] = nc.dram_tensor("d_x2", [P, DC, TQ], f32,
                                     kind="ExternalOutput").ap()
        dbg["d_h2"] = nc.dram_tensor("d_h2", [P, DC, TQ], f32,
                                     kind="ExternalOutput").ap()
        dbg["d_ff1"] = nc.dram_tensor("d_ff1", [P, FC, TQ], f32,
                                      kind="ExternalOutput").ap()
        dbg["d_w1b"] = nc.dram_tensor("d_w1b", [P, DC, DFF], f32,
                                      kind="ExternalOutput").ap()
        dbg["d_w2b"] = nc.dram_tensor("d_w2b", [P, FC, D], f32,
                                      kind="ExternalOutput").ap()

    w_v = {n: w_d[n].rearrange("(c p) o -> p c o", p=P) for n in w_d}
    w1_v = w1_d.rearrange("(c p) o -> p c o", p=P)
    w2_v = w2_d.rearrange("(c p) o -> p c o", p=P)
    hT_v = hT_d.rearrange("(c p) t -> p c t", p=P)
    xres_v = xres_d.rearrange("(c p) t -> p c t", p=P)
    mT_v = mT_d.rearrange("(c p) q -> p c q", p=P)
    outT_v = outT_d.rearrange("(c p) q -> p c q", p=P)

    with tile.TileContext(nc) as tc, ExitStack() as ctx:
        def _emit_body():
            persist = ctx.enter_context(tc.tile_pool(name="persist", bufs=1))

            # --- small persistent vectors (fp32) -----------------------
            vs = {}
            for n in ["bq", "bk", "bo", "b2"]:
                vs[n] = persist.tile([P, DC], f32, tag=f"v_{n}", name=f"v_{n}")
                nc.sync.dma_start(out=vs[n], in_=vec_d[n])
            b1_sb = persist.tile([P, FC], f32, tag="v_b1", name="v_b1")
            nc.sync.dma_start(out=b1_sb, in_=b1_d)
            bvb = persist.tile([P, D], bf, tag="bvb", name="bvb")
            nc.sync.dma_start(out=bvb, in_=bvb_d)
            ones_b = persist.tile([P, 1], bf, tag="ones_b", name="ones_b")
            nc.vector.memset(ones_b, 1.0)
            ones_bf = persist.tile([1, P], bf, tag="ones_bf", name="ones_bf")
            nc.vector.memset(ones_bf, 1.0)
            ones_f = persist.tile([P, 64], f32, tag="ones_f", name="ones_f")
            nc.vector.memset(ones_f, 1.0)

            o_sb = persist.tile([P, DC, TQ], bf, tag="o_sb", name="o_sb")
            x2_sb = persist.tile([P, DC, TQ], f32, tag="x2_sb", name="x2_sb")
            x_res = persist.tile([P, DC, TQ], bf, tag="xres_h2", name="x_res")
            nc.sync.dma_start(out=x_res, in_=xres_v)

            # ---- layernorm stats -> (rstd, mean*rstd) rows ------------
            # feature-major: stats per column via ones-matmuls; rstd via
            # exp(-0.5 ln var) so no Sqrt table set is ever loaded.
            def ln_rows(srcb, sl, W, pool, pspool):
                """srcb [P, DC, *] bf16 -> rows_bf [1, 2, W] bf16:
                row0 = rstd, row1 = mean*rstd."""
                ps_s = pspool.tile([1, W], f32, tag="ln_ps", name="ln_ps_s")
                ps_q = pspool.tile([1, W], f32, tag="ln_ps", name="ln_ps_q")
                for c in range(DC):
                    sq = pool.tile([P, W], bf, tag="lnsq", name="lnsq")
                    nc.vector.tensor_tensor(out=sq, in0=srcb[:, c, sl],
                                            in1=srcb[:, c, sl], op=A.mult)
                    nc.tensor.matmul(ps_s, ones_b, srcb[:, c, sl],
                                     start=(c == 0), stop=(c == DC - 1))
                    nc.tensor.matmul(ps_q, ones_b, sq,
                                     start=(c == 0), stop=(c == DC - 1))
                rows = pool.tile([1, 2, W], f32, tag="lnrows", name="lnrows")
                mean, tmp = rows[:, 0, :], rows[:, 1, :]
                nc.vector.tensor_scalar_mul(mean, ps_s, 1.0 / D)
                # tmp = mean^2 * LNC ; tmp = ps_q*(LNC/D) - tmp ; ln
                nc.vector.scalar_tensor_tensor(
                    out=tmp, in0=mean, scalar=LNC, in1=mean,
                    op0=A.mult, op1=A.mult)
                nc.vector.scalar_tensor_tensor(
                    out=tmp, in0=ps_q, scalar=LNC / D, in1=tmp,
                    op0=A.mult, op1=A.subtract)
                nc.scalar.activation(tmp, tmp, AF.Ln)
                rows_bf = pool.tile([1, 2, W], bf, tag="lnrbf", name="lnrbf")
                nc.scalar.activation(rows_bf[:, 0, :], tmp, AF.Exp,
                                     scale=-0.5)          # rstd
                nc.vector.tensor_tensor(out=rows_bf[:, 1, :], in0=mean,
                                        in1=rows_bf[:, 0, :], op=A.mult)
                return rows_bf

            def ln_norm(srcb, sl, W, dst, rstd_b, mrs_b):
                """dst[:, c, sl] = srcb[:, c, sl]*rstd_b - mrs_b"""
                for c in range(DC):
                    nc.vector.tensor_tensor(out=dst[:, c, sl],
                                            in0=srcb[:, c, sl],
                                            in1=rstd_b, op=A.mult)
                    nc.vector.tensor_tensor(out=dst[:, c, sl],
                                            in0=dst[:, c, sl],
                                            in1=mrs_b, op=A.subtract)

            wo_b = persist.tile([P, DC, D], bf, tag="wo_b", name="wo_b")

            # =============== attention-lifetime pool (right) ============
            with tc.tile_pool(name="pkvq", bufs=1, side="right") as pkvq:
                K_sb = pkvq.tile([P, DC, S], bf, tag="K_sb", name="K_sb")
                V65 = pkvq.tile([P, TC, H, 65], bf, tag="V65", name="V65")
                Q_sb = pkvq.tile([P, DC, TQ], bf, tag="Q_sb", name="Q_sb")
                # ones column of V65 (col 64 of every head slot)
                nc.vector.memset(V65[:, :, :, 64:65], 1.0)

                with tc.tile_pool(name="ph_h", bufs=1, side="right") as ph_h:
                    hT = ph_h.tile([P, DC, S], f8, tag="hT", name="hT")
                    # LN1 is precomputed on the host: hT = z = LN1(x),
                    # fp8; loaded per token-slice so Q/K start early
                    for n in range(NT):
                        sl = slice(n * TQ, n * TQ + TQ)
                        nc.sync.dma_start(out=hT[:, :, sl],
                                          in_=hT_v[:, :, sl])
                    if debug:
                        nc.gpsimd.dma_start(out=dbg["d_hT"], in_=hT)

                    # --- phase C: Q/K/V (fp8 DoubleRow: 256-contraction
                    # per matmul via [128, 2, *] paired chunks) ----------
                    with tc.tile_pool(name="ph_w", bufs=2) as ph_w, \
                         tc.tile_pool(name="ps_mm", bufs=3,
                                      space="PSUM") as ps_mm:

                        def load_w(dram_view):
                            wb = ph_w.tile([P, DC, D], f8, tag="wb",
                                           name="wb")
                            nc.sync.dma_start(out=wb, in_=dram_view)
                            return wb

                        # Q = z.T @ wq' + bq' (1/sqrt(DK), SCH_A folded)
                        wqb = load_w(w_v["wq"])
                        for m in range(DC):
                            ps = ps_mm.tile([P, TQ], f32, tag="mm", name="mm")
                            for cc in range(DC // 2):
                                nc.tensor.matmul(
                                    ps,
                                    wqb[:, 2 * cc:2 * cc + 2,
                                        m * P:(m + 1) * P],
                                    hT[:, 2 * cc:2 * cc + 2, 0:TQ],
                                    start=(cc == 0), stop=(cc == DC // 2 - 1),
                                    perf_mode=DR)
                            nc.scalar.activation(
                                Q_sb[:, m, :], ps, AF.Identity,
                                bias=vs["bq"][:, m:m + 1])
                        # K feature-major [o, t], token-slice-major loop
                        # so attention can start on early slices
                        wkb = load_w(w_v["wk"])
                        for n in range(NT):
                            sl = slice(n * TQ, n * TQ + TQ)
                            for m in range(DC):
                                ps = ps_mm.tile([P, TQ], f32, tag="mm",
                                                name="mm")
                                for cc in range(DC // 2):
                                    nc.tensor.matmul(
                                        ps,
                                        wkb[:, 2 * cc:2 * cc + 2,
                                            m * P:(m + 1) * P],
                                        hT[:, 2 * cc:2 * cc + 2, sl],
                                        start=(cc == 0),
                                        stop=(cc == DC // 2 - 1),
                                        perf_mode=DR)
                                nc.scalar.activation(
                                    K_sb[:, m, sl], ps, AF.Identity,
                                    bias=vs["bk"][:, m:m + 1],
                                    scale=1.0 / WKV_SCALE)
                        # V token-major, all heads: cols 0:64 per slot
                        wvb = load_w(w_v["wv"])
                        for m in range(TC):
                            for half in range(2):
                                hsl = slice(half * 384, half * 384 + 384)
                                h0 = half * 6
                                ps = ps_mm.tile([P, TQ], f32, tag="mm",
                                                name="mm")
                                for cc in range(DC // 2):
                                    nc.tensor.matmul(
                                        ps[:, 0:384],
                                        hT[:, 2 * cc:2 * cc + 2,
                                           m * P:(m + 1) * P],
                                        wvb[:, 2 * cc:2 * cc + 2, hsl],
                                        start=(cc == 0),
                                        stop=(cc == DC // 2 - 1),
                                        perf_mode=DR)
                                pv = ps[:, 0:384].rearrange(
                                    "p (h k) -> p h k", k=64)
                                bv3 = bvb[:, hsl].rearrange(
                                    "p (h k) -> p h k", k=64)
                                nc.vector.scalar_tensor_tensor(
                                    out=V65[:, m, h0:h0 + 6, 0:64],
                                    in0=pv, scalar=1.0 / WKV_SCALE,
                                    in1=bv3, op0=A.mult, op1=A.add)

                # wo + w1 prefetch (overlap attention; used in E/F)
                ph_w12 = ctx.enter_context(
                    tc.tile_pool(name="ph_w12", bufs=1))
                nc.sync.dma_start(out=wo_b, in_=w_v["wo"])
                if debug:
                    nc.gpsimd.dma_start(out=dbg["d_K"], in_=K_sb)
                    nc.gpsimd.dma_start(out=dbg["d_Q"], in_=Q_sb)
                    nc.gpsimd.dma_start(out=dbg["d_V"], in_=V65)
                w1b = ph_w12.tile([P, DC, DFF], bf, tag="w1b", name="w1b")
                for c in range(DC):
                    nc.sync.dma_start(out=w1b[:, c, :], in_=w1_v[:, c, :])

                # ------------- phase D: attention -----------------------
                with tc.tile_pool(name="ph_m", bufs=1) as ph_m, \
                     tc.tile_pool(name="ph_p", bufs=4) as ph_p, \
                     tc.tile_pool(name="ph_nr", bufs=2) as ph_nr, \
                     tc.tile_pool(name="ps_s", bufs=2, space="PSUM") as ps_s, \
                     tc.tile_pool(name="ps_o", bufs=4, space="PSUM") as ps_o:
                    maskb = ph_m.tile([P, TC, TQ], bf, tag="maskb",
                                      name="maskb")
                    nc.sync.dma_start(out=maskb, in_=mT_v)
                    for g4 in range(3):
                        o_ps = [ps_o.tile([P, TQ], f32, tag="o_ps",
                                          name="o_ps") for _ in range(4)]
                        s_tiles = {}
                        for t in range(TC + 1):
                            # stage 1: scores for step t (2 heads per psum
                            # tile; each pair on disjoint PE row groups)
                            if t < TC:
                                for w in range(2):
                                    s_ps = ps_s.tile([P, 2, TQ], f32,
                                                     tag="s_ps", name="s_ps")
                                    for jj in range(2):
                                        h = 4 * g4 + 2 * w + jj
                                        hp = (h % 2) * 64
                                        hc = h // 2
                                        nc.tensor.matmul(
                                            s_ps[:, jj, :],
                                            K_sb[hp:hp + 64, hc,
                                                 t * P:(t + 1) * P],
                                            Q_sb[hp:hp + 64, hc, :],
                                            start=True, stop=True)
                                    s_tiles[(t, w)] = s_ps
                            # stage 2: exp/mask/AV for step t-1.  Scores
                            # arrive pre-scaled by SCH_A (folded into wq):
                            # 2/3 of chunks exp on ACT (scale=1/SCH_A),
                            # 1/3 on DVE via the Schraudolph bf16 bit
                            # trick, balancing the two engines.
                            if t >= 1:
                                tp = t - 1
                                for w in range(2):
                                    s_ps = s_tiles.pop((tp, w))
                                    pt = ph_p.tile([P, 2, TQ], bf, tag="pt",
                                                   name="pt")
                                    if (tp + w) % 2 == 1:
                                        # fused exp+mask: (s+B)*m in f32,
                                        # then the int16 convert IS the
                                        # Schraudolph bf16 bit pattern;
                                        # masked lanes hit exactly 0, so
                                        # the separate mask multiply for
                                        # these tiles disappears.  Half
                                        # the tiles (one per step) take
                                        # this single-hop DVE path, the
                                        # other half the ACT exp path.
                                        nc.vector.scalar_tensor_tensor(
                                            out=pt.bitcast(i16),
                                            in0=s_ps, scalar=SCH_B,
                                            in1=maskb[:, tp:tp + 1, :]
                                            .to_broadcast([P, 2, TQ]),
                                            op0=A.add, op1=A.mult)
                                    else:
                                        nc.scalar.activation(
                                            pt, s_ps, AF.Exp,
                                            scale=1.0 / SCH_A)
                                        nc.vector.tensor_tensor(
                                            out=pt, in0=pt,
                                            in1=maskb[:, tp:tp + 1, :]
                                            .to_broadcast([P, 2, TQ]),
                                            op=A.mult)
                                    for jj in range(2):
                                        j = 2 * w + jj
                                        h = 4 * g4 + j
                                        nc.tensor.matmul(
                                            o_ps[j][0:65, :],
                                            V65[:, tp, h, :],
                                            pt[:, jj, :],
                                            start=(tp == 0),
                                            stop=(tp == TC - 1),
                                            skip_group_check=True)
                        # normalize: o_h *= exp(-ln rowsum_h).  Ln/Exp
                        # live in the same ACT table set as the softmax
                        # exp (natural_log_exp_and_others, see
                        # _patch_act_tables), so no table reloads.
                        # o_sb chunk c packs [head c | head c+6]; wo rows
                        # are host-permuted to match.  Heads 6..11 are
                        # normalized into a staging tile and partition-
                        # shifted to rows 64:128 by a local SBUF->SBUF
                        # DMA.  The ln-rowsum row (psum partition 64) is
                        # broadcast to partitions 0:64 by a ones-matmul.
                        for j in range(4):
                            h = 4 * g4 + j
                            lnr = ph_nr.tile([P, TQ], f32, tag="lnr",
                                             name="lnr")
                            nc.scalar.activation(lnr[64:65, :],
                                                 o_ps[j][64:65, :],
                                                 AF.Ln)
                            rb_ps = ps_s.tile([P, 2, TQ], f32, tag="s_ps",
                                              name="rb_ps")
                            nc.tensor.matmul(rb_ps[0:64, 0, :],
                                             ones_f[64:65, :],
                                             lnr[64:65, :],
                                             start=True, stop=True)
                            rb_sb = ph_nr.tile([P, TQ], bf, tag="rb",
                                               name="rb")
                            nc.scalar.activation(rb_sb[0:64, :],
                                                 rb_ps[0:64, 0, :],
                                                 AF.Exp, scale=-1.0)
                            if h < 6:
                                nc.vector.tensor_tensor(
                                    out=o_sb[0:64, h, :],
                                    in0=o_ps[j][0:64, :], in1=rb_sb[0:64, :],
                                    op=A.mult)
                            else:
                                o_mv = ph_nr.tile([64, TQ], bf, tag="o_mv",
                                                  name="o_mv")
                                nc.vector.tensor_tensor(
                                    out=o_mv, in0=o_ps[j][0:64, :],
                                    in1=rb_sb[0:64, :], op=A.mult)
                                nc.gpsimd.dma_start(
                                    out=o_sb[64:128, h - 6, :], in_=o_mv)

            if debug:
                nc.gpsimd.dma_start(out=dbg["d_osb"], in_=o_sb)

            # w2 prefetch (pkvq space just freed; overlaps phase E)
            w2b = ph_w12.tile([P, FC, D], bf, tag="w2b", name="w2b")
            for c in range(0, FC, 4):
                nc.sync.dma_start(out=w2b[:, c:c + 4, :],
                                  in_=w2_v[:, c:c + 4, :])

            # ---------- phase E: o-proj + residual + LN2 ----------------
            with tc.tile_pool(name="ph_e", bufs=1) as ph_e, \
                 tc.tile_pool(name="lnp2", bufs=2) as lnp2, \
                 tc.tile_pool(name="ps_e", bufs=3, space="PSUM") as ps_e:
                for m in range(DC):
                    ps = ps_e.tile([P, TQ], f32, tag="mm2", name="mm2")
                    for c in range(DC):
                        nc.tensor.matmul(
                            ps, wo_b[:, c, m * P:(m + 1) * P], o_sb[:, c, :],
                            start=(c == 0), stop=(c == DC - 1))
                    # x2 = (ps + bo) + x   (one fused pass)
                    nc.vector.scalar_tensor_tensor(
                        out=x2_sb[:, m, :], in0=ps,
                        scalar=vs["bo"][:, m:m + 1],
                        in1=x_res[:, m, :], op0=A.add, op1=A.add)
                # x2 in bf16 for LN2 stats + normalization source
                x2b = ph_e.tile([P, DC, TQ], bf, tag="x2b", name="x2b")
                for c in range(DC):
                    nc.vector.tensor_copy(out=x2b[:, c, :],
                                          in_=x2_sb[:, c, :])
                rows_bf = ln_rows(x2b, slice(0, TQ), TQ, lnp2, ps_e)
                # broadcast rstd/mrs via ones-matmul (critical path:
                # lower latency than a gpsimd broadcast)
                rs_ps = ps_e.tile([P, TQ], f32, tag="mm2", name="rs_ps")
                ms_ps = ps_e.tile([P, TQ], f32, tag="mm2", name="ms_ps")
                nc.tensor.matmul(rs_ps, ones_bf, rows_bf[:, 0, :],
                                 start=True, stop=True)
                nc.tensor.matmul(ms_ps, ones_bf, rows_bf[:, 1, :],
                                 start=True, stop=True)
                rstd_b = lnp2.tile([P, TQ], bf, tag="ln2rb", name="ln2rb")
                mrs_b = lnp2.tile([P, TQ], bf, tag="ln2mb", name="ln2mb")
                nc.vector.tensor_copy(out=rstd_b, in_=rs_ps)
                nc.vector.tensor_copy(out=mrs_b, in_=ms_ps)
                h2 = persist.tile([P, DC, TQ], bf, tag="xres_h2", name="h2")
                ln_norm(x2b, slice(0, TQ), TQ, h2, rstd_b, mrs_b)
                if debug:
                    nc.sync.dma_start(out=dbg["d_x2"], in_=x2_sb)
                    nc.gpsimd.dma_start(out=dbg["d_h2"], in_=h2)
                    nc.gpsimd.dma_start(out=dbg["d_rb"], in_=rstd_b)

            # ---------------- phase F: FFN ------------------------------
            with tc.tile_pool(name="ph_f", bufs=1) as ph_f, \
                 tc.tile_pool(name="ph_oc", bufs=2) as ph_oc, \
                 tc.tile_pool(name="ps_f", bufs=3, space="PSUM") as ps_f:
                ff1 = ph_f.tile([P, FC, TQ], bf, tag="ff1", name="ff1")
                for m in range(FC):
                    ps = ps_f.tile([P, TQ], f32, tag="mmf", name="mmf")
                    for c in range(DC):
                        nc.tensor.matmul(ps, w1b[:, c, m * P:(m + 1) * P],
                                         h2[:, c, :],
                                         start=(c == 0), stop=(c == DC - 1))
                    nc.scalar.activation(ff1[:, m, :], ps, AF.Relu,
                                         bias=b1_sb[:, m:m + 1])
                if debug:
                    nc.gpsimd.dma_start(out=dbg["d_ff1"], in_=ff1)
                    nc.gpsimd.dma_start(out=dbg["d_w1b"], in_=w1b)
                    nc.gpsimd.dma_start(out=dbg["d_w2b"], in_=w2b)
                for m in range(DC):
                    ps = ps_f.tile([P, TQ], f32, tag="mmf", name="mmf")
                    for c in range(FC):
                        nc.tensor.matmul(
                            ps, w2b[:, c, m * P:(m + 1) * P], ff1[:, c, :],
                            start=(c == 0), stop=(c == FC - 1))
                    out_c = ph_oc.tile([P, TQ], f32, tag="out_c",
                                       name="out_c")
                    nc.vector.scalar_tensor_tensor(
                        out=out_c, in0=ps, scalar=vs["b2"][:, m:m + 1],
                        in1=x2_sb[:, m, :], op0=A.add, op1=A.add)
                    nc.sync.dma_start(out=outT_v[:, m, :], in_=out_c)

        if bench_iters:
            with tc.For_i(0, bench_iters, 1):
                _emit_body()
        else:
            _emit_body()
    nc.compile()
    return nc


def _get_module():
    if "nc" not in _CACHE:
        _CACHE["nc"] = _build_module()
    return _CACHE["nc"]


def _make_in_maps(inputs):
    bf16 = ml_dtypes.bfloat16
    f32 = np.float32
    x = np.asarray(inputs["x"], f32)
    mask = np.asarray(inputs["mask"])
    a1 = np.asarray(inputs["alpha1"], f32)
    b1n = np.asarray(inputs["beta1"], f32)
    a2 = np.asarray(inputs["alpha2"], f32)
    b2n = np.asarray(inputs["beta2"], f32)
    wq = np.asarray(inputs["wq"], f32)
    wk = np.asarray(inputs["wk"], f32)
    wv = np.asarray(inputs["wv"], f32)
    wo = np.asarray(inputs["wo"], f32)
    w1 = np.asarray(inputs["w1"], f32)
    w2 = np.asarray(inputs["w2"], f32)
    bq = np.asarray(inputs["bq"], f32)
    bk = np.asarray(inputs["bk"], f32)
    bv = np.asarray(inputs["bv"], f32)
    bo = np.asarray(inputs["bo"], f32)
    b1 = np.asarray(inputs["b1"], f32)
    b2 = np.asarray(inputs["b2"], f32)

    sc = SCH_A / float(np.sqrt(DK))
    # fold LN1 affine into wq/wk/wv; 1/sqrt(DK) and the Schraudolph
    # scale SCH_A into wq/bq; the fp8 range scale WKV_SCALE into wk/wv
    # (undone on the psum).
    wq_f = (a1[:, None] * wq) * sc
    bq_f = (bq + b1n @ wq) * sc
    wk_f = (a1[:, None] * wk) * WKV_SCALE
    bk_f = bk + b1n @ wk
    wv_f = (a1[:, None] * wv) * WKV_SCALE
    bv_f = bv + b1n @ wv
    # fold LN2 affine into w1
    w1_f = a2[:, None] * w1
    b1_f = b1 + b2n @ w1

    # o_sb chunk c holds [head c (rows 0:64) | head c+6 (rows 64:128)]:
    # permute wo's rows to match.
    rperm = np.concatenate(
        [np.r_[c * DK:(c + 1) * DK, (c + 6) * DK:(c + 7) * DK]
         for c in range(6)])
    wo_p = wo[rperm, :]

    fp8 = ml_dtypes.float8_e4m3
    full = {
        "wq": np.ascontiguousarray(wq_f.astype(fp8)),
        "wk": np.ascontiguousarray(wk_f.astype(fp8)),
        "wv": np.ascontiguousarray(wv_f.astype(fp8)),
        "wo": np.ascontiguousarray(wo_p.astype(bf16)),
        "w1": np.ascontiguousarray(w1_f.astype(bf16)),
        "w2": np.ascontiguousarray(w2.astype(bf16)),
        "bvb": np.ascontiguousarray(
            np.broadcast_to(bv_f.astype(bf16), (P, D))),
    }
    for n, v in [("bq", bq_f), ("bk", bk_f), ("bo", bo), ("b2", b2)]:
        full[n] = np.ascontiguousarray(v.reshape(DC, P).T)
    full["b1"] = np.ascontiguousarray(b1_f.reshape(FC, P).T)

    # LN1 precomputed on the host (pure function of the input x)
    mu = x.mean(-1, keepdims=True)
    var = ((x - mu) ** 2).sum(-1, keepdims=True) / (D - 1)
    z = (x - mu) / (np.sqrt(var) + 1e-6)

    in_maps = []
    for c in range(NCORES):
        b, r = divmod(c, 4)
        hT = np.ascontiguousarray(
            np.roll(z[b].T, -TQ * r, axis=1).astype(fp8))
        xres = np.ascontiguousarray(
            x[b, TQ * r:TQ * r + TQ, :].T.astype(bf16))
        mT = np.ascontiguousarray(
            np.roll(mask[0, 0, TQ * r:TQ * r + TQ, :].T.astype(f32),
                    -TQ * r, axis=0).astype(bf16))
        m = dict(full)
        m["hT"] = hT
        m["xres"] = xres
        m["maskT"] = mT
        in_maps.append(m)
    return in_maps


def kernel(**inputs):
    nc = _get_module()
    in_maps = _make_in_maps(inputs)
    from concourse import bass2jax
    results = bass2jax.run_bass_via_pjrt(nc, in_maps, n_cores=NCORES)
    out = np.empty((B, S, D), np.float32)
    for c in range(NCORES):
        b, r = divmod(c, 4)
        out[b, TQ * r:TQ * r + TQ, :] = results[c]["outT"].T
    return out



# revision 31
# speedup vs baseline: 1.3692x; 1.3692x over previous
# Trainium2 Bass kernel for a pre-norm transformer encoder block.
#
# Sharding: 8 cores = 2 batches x 4 query-blocks of 512 tokens.
# Each core recomputes K/V for its batch (replicated 4x within the batch
# group), which removes every collective; FFN/projection rows are
# disjoint. The per-core query-block offset cannot appear in the
# SPMD-identical program, so the host *rolls* each core's token axis to
# put its query block at columns 0..511 — attention is permutation-
# equivariant over keys when the mask rows are rolled identically.
#
# Host-side prep: all big tensors are cast to bf16; the LayerNorm affine
# (alpha, beta) is folded into the following weight matrices and biases
# (exact algebra), and 1/sqrt(DK) is folded into wq/bq.  On-chip LN
# therefore produces the plain normalized z.  wv's columns are permuted
# so the V projection emits heads in [evens, odds] order (see V65).
#
# The softmax row-sums ride along the attention-output matmul as a ones
# column appended to V (even heads: ones col 64 -> psum row 64 with o at
# rows 0:64; odd heads: ones col 0 -> psum row 63 with o at rows 64:128,
# matching the feature-major o_sb layout).  1/rowsum is computed as
# exp(-ln r) on the scalar engine with a gpsimd partition_broadcast in
# between; LN rstd likewise uses exp(-0.5 ln var).  Ln/Exp share one ACT
# table set with the softmax exp, so no table reload ever happens.

import numpy as np
import ml_dtypes

B, S, D, H, DK, DFF = 2, 2048, 768, 12, 64, 3072
P = 128
TQ = 512               # query rows per core
NCORES = 8
DC = D // P            # 6  d-chunks
FC = DFF // P          # 24 dff-chunks
TC = S // P            # 16 key chunks
NT = S // TQ           # 4  512-wide token slices
LNC = float(D) / (D - 1)   # ddof=1 correction
SCH_A = 128.0 / float(np.log(2.0))   # Schraudolph bf16 scale (folded into wq)
SCH_B = 16256.0 - 7.42               # Schraudolph bf16 bias
WKV_SCALE = 64.0                     # fp8 scale for wk/wv (undone on psum)

_CACHE: dict = {}


def _patch_act_tables():
    """Restrict the ACT table-set chooser to natural_log_exp_and_others
    (contains every activation this kernel uses: exp/ln/relu/copy/
    identity/square), so exactly one ACT_TABLE_LOAD is ever emitted
    instead of thrashing between the exp and ln sets per instruction.
    Set ids are preserved (other sets stay in the dict, just emptied)."""
    import concourse.bacc as bacc_mod
    if getattr(bacc_mod, "_act_tables_patched", False):
        return
    orig = bacc_mod.get_activation_tables

    def _gat(arch):
        t = orig(arch)
        if "natural_log_exp_and_others" in t:
            return {k: (v if k == "natural_log_exp_and_others" else set())
                    for k, v in t.items()}
        return t

    bacc_mod.get_activation_tables = _gat
    bacc_mod._act_tables_patched = True


def _build_module(bench_iters=0, debug=False):
    import concourse.bass as bass
    import concourse.mybir as mybir
    import concourse.tile as tile
    from concourse import bacc
    from contextlib import ExitStack

    _patch_act_tables()

    f32 = mybir.dt.float32
    bf = mybir.dt.bfloat16
    f8 = mybir.dt.float8e4
    i16 = mybir.dt.int16
    A = mybir.AluOpType
    AF = mybir.ActivationFunctionType
    DR = mybir.MatmulPerfMode.DoubleRow

    nc = bacc.Bacc(trn_type="TRN2", num_swdge_queues=4)

    def din(name, shape, dt=bf):
        return nc.dram_tensor(name, shape, dt, kind="ExternalInput").ap()

    hT_d = din("hT", [D, S], f8)    # LN1(x) precomputed on host, fp8
    xres_d = din("xres", [D, TQ])   # residual slice of x
    mT_d = din("maskT", [S, TQ])
    w_d = {n: din(n, [D, D], f8 if n in ("wq", "wk", "wv") else bf)
           for n in ["wq", "wk", "wv", "wo"]}
    w1_d = din("w1", [D, DFF])
    w2_d = din("w2", [DFF, D])
    vec_d = {n: din(n, [P, DC], f32) for n in ["bq", "bk", "bo", "b2"]}
    b1_d = din("b1", [P, FC], f32)
    bvb_d = din("bvb", [P, D])
    outT_d = nc.dram_tensor("outT", [D, TQ], f32, kind="ExternalOutput").ap()
    dbg = {}
    if debug:
        for n, shp in [("d_hT", [P, DC, S]), ("d_K", [P, DC, S]),
                       ("d_Q", [P, DC, TQ]), ("d_V", [P, TC, H, 65]),
                       ("d_osb", [P, DC, TQ]), ("d_rb", [P, TQ])]:
            dbg[n] = nc.dram_tensor(n, shp, mybir.dt.float32,
                                    kind="ExternalOutput").ap()
        dbg["d_x2"] = nc.dram_tensor("d_x2", [P, DC, TQ], f32,
                                     kind="ExternalOutput").ap()
        dbg["d_h2"] = nc.dram_tensor("d_h2", [P, DC, TQ], f32,
                                     kind="ExternalOutput").ap()
        dbg["d_ff1"] = nc.dram_tensor("d_ff1", [P, FC, TQ], f32,
                                      kind="ExternalOutput").ap()
        dbg["d_w1b"] = nc.dram_tensor("d_w1b", [P, DC, DFF], f32,
                                      kind="ExternalOutput").ap()
        dbg["d_w2b"] = nc.dram_tensor("d_w2b", [P, FC, D], f32,
                                      kind="ExternalOutput").ap()

    w_v = {n: w_d[n].rearrange("(c p) o -> p c o", p=P) for n in w_d}
    w1_v = w1_d.rearrange("(c p) o -> p c o", p=P)
    w2_v = w2_d.rearrange("(c p) o -> p c o", p=P)
    hT_v = hT_d.rearrange("(c p) t -> p c t", p=P)
    xres_v = xres_d.rearrange("(c p) t -> p c t", p=P)
    mT_v = mT_d.rearrange("(c p) q -> p c q", p=P)
    outT_v = outT_d.rearrange("(c p) q -> p c q", p=P)

    with tile.TileContext(nc) as tc, ExitStack() as ctx:
        def _emit_body():
            persist = ctx.enter_context(tc.tile_pool(name="persist", bufs=1))

            # --- small persistent vectors (fp32) -----------------------
            vs = {}
            for n in ["bq", "bk", "bo", "b2"]:
                vs[n] = persist.tile([P, DC], f32, tag=f"v_{n}", name=f"v_{n}")
                nc.sync.dma_start(out=vs[n], in_=vec_d[n])
            b1_sb = persist.tile([P, FC], f32, tag="v_b1", name="v_b1")
            nc.sync.dma_start(out=b1_sb, in_=b1_d)
            bvb = persist.tile([P, D], bf, tag="bvb", name="bvb")
            nc.sync.dma_start(out=bvb, in_=bvb_d)
            ones_b = persist.tile([P, 1], bf, tag="ones_b", name="ones_b")
            nc.vector.memset(ones_b, 1.0)
            ones_bf = persist.tile([1, P], bf, tag="ones_bf", name="ones_bf")
            nc.vector.memset(ones_bf, 1.0)
            ones_f = persist.tile([P, 64], f32, tag="ones_f", name="ones_f")
            nc.vector.memset(ones_f, 1.0)

            o_sb = persist.tile([P, DC, TQ], bf, tag="o_sb", name="o_sb")
            x2_sb = persist.tile([P, DC, TQ], f32, tag="x2_sb", name="x2_sb")
            x_res = persist.tile([P, DC, TQ], bf, tag="xres_h2", name="x_res")
            nc.sync.dma_start(out=x_res, in_=xres_v)

            # ---- layernorm stats -> (rstd, mean*rstd) rows ------------
            # feature-major: stats per column via ones-matmuls; rstd via
            # exp(-0.5 ln var) so no Sqrt table set is ever loaded.
            def ln_rows(srcb, sl, W, pool, pspool):
                """srcb [P, DC, *] bf16 -> rows_bf [1, 2, W] bf16:
                row0 = rstd, row1 = mean*rstd."""
                ps_s = pspool.tile([1, W], f32, tag="ln_ps", name="ln_ps_s")
                ps_q = pspool.tile([1, W], f32, tag="ln_ps", name="ln_ps_q")
                for c in range(DC):
                    sq = pool.tile([P, W], bf, tag="lnsq", name="lnsq")
                    nc.vector.tensor_tensor(out=sq, in0=srcb[:, c, sl],
                                            in1=srcb[:, c, sl], op=A.mult)
                    nc.tensor.matmul(ps_s, ones_b, srcb[:, c, sl],
                                     start=(c == 0), stop=(c == DC - 1))
                    nc.tensor.matmul(ps_q, ones_b, sq,
                                     start=(c == 0), stop=(c == DC - 1))
                rows = pool.tile([1, 2, W], f32, tag="lnrows", name="lnrows")
                mean, tmp = rows[:, 0, :], rows[:, 1, :]
                nc.vector.tensor_scalar_mul(mean, ps_s, 1.0 / D)
                # tmp = mean^2 * LNC ; tmp = ps_q*(LNC/D) - tmp ; ln
                nc.vector.scalar_tensor_tensor(
                    out=tmp, in0=mean, scalar=LNC, in1=mean,
                    op0=A.mult, op1=A.mult)
                nc.vector.scalar_tensor_tensor(
                    out=tmp, in0=ps_q, scalar=LNC / D, in1=tmp,
                    op0=A.mult, op1=A.subtract)
                nc.scalar.activation(tmp, tmp, AF.Ln)
                rows_bf = pool.tile([1, 2, W], bf, tag="lnrbf", name="lnrbf")
                nc.scalar.activation(rows_bf[:, 0, :], tmp, AF.Exp,
                                     scale=-0.5)          # rstd
                nc.vector.tensor_tensor(out=rows_bf[:, 1, :], in0=mean,
                                        in1=rows_bf[:, 0, :], op=A.mult)
                return rows_bf

            def ln_norm(srcb, sl, W, dst, rstd_b, mrs_b):
                """dst[:, c, sl] = srcb[:, c, sl]*rstd_b - mrs_b"""
                for c in range(DC):
                    nc.vector.tensor_tensor(out=dst[:, c, sl],
                                            in0=srcb[:, c, sl],
                                            in1=rstd_b, op=A.mult)
                    nc.vector.tensor_tensor(out=dst[:, c, sl],
                                            in0=dst[:, c, sl],
                                            in1=mrs_b, op=A.subtract)

            wo_b = persist.tile([P, DC, D], bf, tag="wo_b", name="wo_b")

            # =============== attention-lifetime pool (right) ============
            with tc.tile_pool(name="pkvq", bufs=1, side="right") as pkvq:
                K_sb = pkvq.tile([P, DC, S], bf, tag="K_sb", name="K_sb")
                V65 = pkvq.tile([P, TC, H, 65], bf, tag="V65", name="V65")
                Q_sb = pkvq.tile([P, DC, TQ], bf, tag="Q_sb", name="Q_sb")
                # ones column of V65 (col 64 of every head slot)
                nc.vector.memset(V65[:, :, :, 64:65], 1.0)

                with tc.tile_pool(name="ph_h", bufs=1, side="right") as ph_h:
                    hT = ph_h.tile([P, DC, S], f8, tag="hT", name="hT")
                    # LN1 is precomputed on the host: hT = z = LN1(x),
                    # fp8; loaded per token-slice so Q/K start early
                    for n in range(NT):
                        sl = slice(n * TQ, n * TQ + TQ)
                        nc.sync.dma_start(out=hT[:, :, sl],
                                          in_=hT_v[:, :, sl])
                    if debug:
                        nc.gpsimd.dma_start(out=dbg["d_hT"], in_=hT)

                    # --- phase C: Q/K/V (fp8 DoubleRow: 256-contraction
                    # per matmul via [128, 2, *] paired chunks) ----------
                    with tc.tile_pool(name="ph_w", bufs=2) as ph_w, \
                         tc.tile_pool(name="ps_mm", bufs=3,
                                      space="PSUM") as ps_mm:

                        def load_w(dram_view):
                            wb = ph_w.tile([P, DC, D], f8, tag="wb",
                                           name="wb")
                            nc.sync.dma_start(out=wb, in_=dram_view)
                            return wb

                        # Q = z.T @ wq' + bq' (1/sqrt(DK), SCH_A folded)
                        wqb = load_w(w_v["wq"])
                        for m in range(DC):
                            ps = ps_mm.tile([P, TQ], f32, tag="mm", name="mm")
                            for cc in range(DC // 2):
                                nc.tensor.matmul(
                                    ps,
                                    wqb[:, 2 * cc:2 * cc + 2,
                                        m * P:(m + 1) * P],
                                    hT[:, 2 * cc:2 * cc + 2, 0:TQ],
                                    start=(cc == 0), stop=(cc == DC // 2 - 1),
                                    perf_mode=DR)
                            nc.scalar.activation(
                                Q_sb[:, m, :], ps, AF.Identity,
                                bias=vs["bq"][:, m:m + 1])
                        # K feature-major [o, t], token-slice-major loop
                        # so attention can start on early slices
                        wkb = load_w(w_v["wk"])
                        for n in range(NT):
                            sl = slice(n * TQ, n * TQ + TQ)
                            for m in range(DC):
                                ps = ps_mm.tile([P, TQ], f32, tag="mm",
                                                name="mm")
                                for cc in range(DC // 2):
                                    nc.tensor.matmul(
                                        ps,
                                        wkb[:, 2 * cc:2 * cc + 2,
                                            m * P:(m + 1) * P],
                                        hT[:, 2 * cc:2 * cc + 2, sl],
                                        start=(cc == 0),
                                        stop=(cc == DC // 2 - 1),
                                        perf_mode=DR)
                                nc.scalar.activation(
                                    K_sb[:, m, sl], ps, AF.Identity,
                                    bias=vs["bk"][:, m:m + 1],
                                    scale=1.0 / WKV_SCALE)
                        # V token-major, all heads: cols 0:64 per slot
                        wvb = load_w(w_v["wv"])
                        for m in range(TC):
                            for half in range(2):
                                hsl = slice(half * 384, half * 384 + 384)
                                h0 = half * 6
                                ps = ps_mm.tile([P, TQ], f32, tag="mm",
                                                name="mm")
                                for cc in range(DC // 2):
                                    nc.tensor.matmul(
                                        ps[:, 0:384],
                                        hT[:, 2 * cc:2 * cc + 2,
                                           m * P:(m + 1) * P],
                                        wvb[:, 2 * cc:2 * cc + 2, hsl],
                                        start=(cc == 0),
                                        stop=(cc == DC // 2 - 1),
                                        perf_mode=DR)
                                pv = ps[:, 0:384].rearrange(
                                    "p (h k) -> p h k", k=64)
                                bv3 = bvb[:, hsl].rearrange(
                                    "p (h k) -> p h k", k=64)
                                nc.vector.scalar_tensor_tensor(
                                    out=V65[:, m, h0:h0 + 6, 0:64],
                                    in0=pv, scalar=1.0 / WKV_SCALE,
                                    in1=bv3, op0=A.mult, op1=A.add)

                # wo + w1 prefetch (overlap attention; used in E/F)
                ph_w12 = ctx.enter_context(
                    tc.tile_pool(name="ph_w12", bufs=1))
                nc.sync.dma_start(out=wo_b, in_=w_v["wo"])
                if debug:
                    nc.gpsimd.dma_start(out=dbg["d_K"], in_=K_sb)
                    nc.gpsimd.dma_start(out=dbg["d_Q"], in_=Q_sb)
                    nc.gpsimd.dma_start(out=dbg["d_V"], in_=V65)
                w1b = ph_w12.tile([P, DC, DFF], bf, tag="w1b", name="w1b")
                for c in range(DC):
                    nc.sync.dma_start(out=w1b[:, c, :], in_=w1_v[:, c, :])

                # ------------- phase D: attention -----------------------
                with tc.tile_pool(name="ph_m", bufs=1) as ph_m, \
                     tc.tile_pool(name="ph_p", bufs=4) as ph_p, \
                     tc.tile_pool(name="ph_nr", bufs=2) as ph_nr, \
                     tc.tile_pool(name="ps_s", bufs=2, space="PSUM") as ps_s, \
                     tc.tile_pool(name="ps_o", bufs=4, space="PSUM") as ps_o:
                    maskb = ph_m.tile([P, TC, TQ], bf, tag="maskb",
                                      name="maskb")
                    nc.sync.dma_start(out=maskb, in_=mT_v)
                    for g4 in range(3):
                        o_ps = [ps_o.tile([P, TQ], f32, tag="o_ps",
                                          name="o_ps") for _ in range(4)]
                        s_tiles = {}
                        for t in range(TC + 1):
                            # stage 1: scores for step t (2 heads per psum
                            # tile; each pair on disjoint PE row groups)
                            if t < TC:
                                for w in range(2):
                                    s_ps = ps_s.tile([P, 2, TQ], f32,
                                                     tag="s_ps", name="s_ps")
                                    for jj in range(2):
                                        h = 4 * g4 + 2 * w + jj
                                        hp = (h % 2) * 64
                                        hc = h // 2
                                        nc.tensor.matmul(
                                            s_ps[:, jj, :],
                                            K_sb[hp:hp + 64, hc,
                                                 t * P:(t + 1) * P],
                                            Q_sb[hp:hp + 64, hc, :],
                                            start=True, stop=True)
                                    s_tiles[(t, w)] = s_ps
                            # stage 2: exp/mask/AV for step t-1.  Scores
                            # arrive pre-scaled by SCH_A (folded into wq):
                            # 2/3 of chunks exp on ACT (scale=1/SCH_A),
                            # 1/3 on DVE via the Schraudolph bf16 bit
                            # trick, balancing the two engines.
                            if t >= 1:
                                tp = t - 1
                                for w in range(2):
                                    s_ps = s_tiles.pop((tp, w))
                                    pt = ph_p.tile([P, 2, TQ], bf, tag="pt",
                                                   name="pt")
                                    if tp % 2 == 1:
                                        # fused exp+mask: (s+B)*m in f32,
                                        # then the int16 convert IS the
                                        # Schraudolph bf16 bit pattern;
                                        # masked lanes hit exactly 0, so
                                        # the separate mask multiply for
                                        # these tiles disappears.  Whole
                                        # steps only: both tiles of a
                                        # step take the same path, so the
                                        # in-order PE queue never waits
                                        # on a slow tile behind a fast
                                        # one.
                                        nc.vector.scalar_tensor_tensor(
                                            out=pt.bitcast(i16),
                                            in0=s_ps, scalar=SCH_B,
                                            in1=maskb[:, tp:tp + 1, :]
                                            .to_broadcast([P, 2, TQ]),
                                            op0=A.add, op1=A.mult)
                                    else:
                                        nc.scalar.activation(
                                            pt, s_ps, AF.Exp,
                                            scale=1.0 / SCH_A)
                                        nc.vector.tensor_tensor(
                                            out=pt, in0=pt,
                                            in1=maskb[:, tp:tp + 1, :]
                                            .to_broadcast([P, 2, TQ]),
                                            op=A.mult)
                                    for jj in range(2):
                                        j = 2 * w + jj
                                        h = 4 * g4 + j
                                        nc.tensor.matmul(
                                            o_ps[j][0:65, :],
                                            V65[:, tp, h, :],
                                            pt[:, jj, :],
                                            start=(tp == 0),
                                            stop=(tp == TC - 1),
                                            skip_group_check=True)
                        # normalize: o_h *= exp(-ln rowsum_h).  Ln/Exp
                        # live in the same ACT table set as the softmax
                        # exp (natural_log_exp_and_others, see
                        # _patch_act_tables), so no table reloads.
                        # o_sb chunk c packs [head c | head c+6]; wo rows
                        # are host-permuted to match.  Heads 6..11 are
                        # normalized into a staging tile and partition-
                        # shifted to rows 64:128 by a local SBUF->SBUF
                        # DMA.  The ln-rowsum row (psum partition 64) is
                        # broadcast to partitions 0:64 by a ones-matmul.
                        for j in range(4):
                            h = 4 * g4 + j
                            lnr = ph_nr.tile([P, TQ], f32, tag="lnr",
                                             name="lnr")
                            nc.scalar.activation(lnr[64:65, :],
                                                 o_ps[j][64:65, :],
                                                 AF.Ln)
                            rb_ps = ps_s.tile([P, 2, TQ], f32, tag="s_ps",
                                              name="rb_ps")
                            nc.tensor.matmul(rb_ps[0:64, 0, :],
                                             ones_f[64:65, :],
                                             lnr[64:65, :],
                                             start=True, stop=True)
                            rb_sb = ph_nr.tile([P, TQ], bf, tag="rb",
                                               name="rb")
                            nc.scalar.activation(rb_sb[0:64, :],
                                                 rb_ps[0:64, 0, :],
                                                 AF.Exp, scale=-1.0)
                            if h < 6:
                                nc.vector.tensor_tensor(
                                    out=o_sb[0:64, h, :],
                                    in0=o_ps[j][0:64, :], in1=rb_sb[0:64, :],
                                    op=A.mult)
                            else:
                                o_mv = ph_nr.tile([64, TQ], bf, tag="o_mv",
                                                  name="o_mv")
                                nc.vector.tensor_tensor(
                                    out=o_mv, in0=o_ps[j][0:64, :],
                                    in1=rb_sb[0:64, :], op=A.mult)
                                nc.gpsimd.dma_start(
                                    out=o_sb[64:128, h - 6, :], in_=o_mv)

            if debug:
                nc.gpsimd.dma_start(out=dbg["d_osb"], in_=o_sb)

            # w2 prefetch (pkvq space just freed; overlaps phase E)
            w2b = ph_w12.tile([P, FC, D], bf, tag="w2b", name="w2b")
            for c in range(0, FC, 4):
                nc.sync.dma_start(out=w2b[:, c:c + 4, :],
                                  in_=w2_v[:, c:c + 4, :])

            # ---------- phase E: o-proj + residual + LN2 ----------------
            with tc.tile_pool(name="ph_e", bufs=1) as ph_e, \
                 tc.tile_pool(name="lnp2", bufs=2) as lnp2, \
                 tc.tile_pool(name="ps_e", bufs=3, space="PSUM") as ps_e:
                for m in range(DC):
                    ps = ps_e.tile([P, TQ], f32, tag="mm2", name="mm2")
                    for c in range(DC):
                        nc.tensor.matmul(
                            ps, wo_b[:, c, m * P:(m + 1) * P], o_sb[:, c, :],
                            start=(c == 0), stop=(c == DC - 1))
                    # x2 = (ps + bo) + x   (one fused pass)
                    nc.vector.scalar_tensor_tensor(
                        out=x2_sb[:, m, :], in0=ps,
                        scalar=vs["bo"][:, m:m + 1],
                        in1=x_res[:, m, :], op0=A.add, op1=A.add)
                # x2 in bf16 for LN2 stats + normalization source
                x2b = ph_e.tile([P, DC, TQ], bf, tag="x2b", name="x2b")
                for c in range(DC):
                    nc.vector.tensor_copy(out=x2b[:, c, :],
                                          in_=x2_sb[:, c, :])
                rows_bf = ln_rows(x2b, slice(0, TQ), TQ, lnp2, ps_e)
                # broadcast rstd/mrs via ones-matmul (critical path:
                # lower latency than a gpsimd broadcast)
                rs_ps = ps_e.tile([P, TQ], f32, tag="mm2", name="rs_ps")
                ms_ps = ps_e.tile([P, TQ], f32, tag="mm2", name="ms_ps")
                nc.tensor.matmul(rs_ps, ones_bf, rows_bf[:, 0, :],
                                 start=True, stop=True)
                nc.tensor.matmul(ms_ps, ones_bf, rows_bf[:, 1, :],
                                 start=True, stop=True)
                rstd_b = lnp2.tile([P, TQ], bf, tag="ln2rb", name="ln2rb")
                mrs_b = lnp2.tile([P, TQ], bf, tag="ln2mb", name="ln2mb")
                nc.vector.tensor_copy(out=rstd_b, in_=rs_ps)
                nc.vector.tensor_copy(out=mrs_b, in_=ms_ps)
                h2 = persist.tile([P, DC, TQ], bf, tag="xres_h2", name="h2")
                ln_norm(x2b, slice(0, TQ), TQ, h2, rstd_b, mrs_b)
                if debug:
                    nc.sync.dma_start(out=dbg["d_x2"], in_=x2_sb)
                    nc.gpsimd.dma_start(out=dbg["d_h2"], in_=h2)
                    nc.gpsimd.dma_start(out=dbg["d_rb"], in_=rstd_b)

            # ---------------- phase F: FFN ------------------------------
            with tc.tile_pool(name="ph_f", bufs=1) as ph_f, \
                 tc.tile_pool(name="ph_oc", bufs=2) as ph_oc, \
                 tc.tile_pool(name="ps_f", bufs=3, space="PSUM") as ps_f:
                ff1 = ph_f.tile([P, FC, TQ], bf, tag="ff1", name="ff1")
                for m in range(FC):
                    ps = ps_f.tile([P, TQ], f32, tag="mmf", name="mmf")
                    for c in range(DC):
                        nc.tensor.matmul(ps, w1b[:, c, m * P:(m + 1) * P],
                                         h2[:, c, :],
                                         start=(c == 0), stop=(c == DC - 1))
                    nc.scalar.activation(ff1[:, m, :], ps, AF.Relu,
                                         bias=b1_sb[:, m:m + 1])
                if debug:
                    nc.gpsimd.dma_start(out=dbg["d_ff1"], in_=ff1)
                    nc.gpsimd.dma_start(out=dbg["d_w1b"], in_=w1b)
                    nc.gpsimd.dma_start(out=dbg["d_w2b"], in_=w2b)
                for m in range(DC):
                    ps = ps_f.tile([P, TQ], f32, tag="mmf", name="mmf")
                    for c in range(FC):
                        nc.tensor.matmul(
                            ps, w2b[:, c, m * P:(m + 1) * P], ff1[:, c, :],
                            start=(c == 0), stop=(c == FC - 1))
                    out_c = ph_oc.tile([P, TQ], f32, tag="out_c",
                                       name="out_c")
                    nc.vector.scalar_tensor_tensor(
                        out=out_c, in0=ps, scalar=vs["b2"][:, m:m + 1],
                        in1=x2_sb[:, m, :], op0=A.add, op1=A.add)
                    nc.sync.dma_start(out=outT_v[:, m, :], in_=out_c)

        if bench_iters:
            with tc.For_i(0, bench_iters, 1):
                _emit_body()
        else:
            _emit_body()
    nc.compile()
    return nc


def _get_module():
    if "nc" not in _CACHE:
        _CACHE["nc"] = _build_module()
    return _CACHE["nc"]


def _make_in_maps(inputs):
    bf16 = ml_dtypes.bfloat16
    f32 = np.float32
    x = np.asarray(inputs["x"], f32)
    mask = np.asarray(inputs["mask"])
    a1 = np.asarray(inputs["alpha1"], f32)
    b1n = np.asarray(inputs["beta1"], f32)
    a2 = np.asarray(inputs["alpha2"], f32)
    b2n = np.asarray(inputs["beta2"], f32)
    wq = np.asarray(inputs["wq"], f32)
    wk = np.asarray(inputs["wk"], f32)
    wv = np.asarray(inputs["wv"], f32)
    wo = np.asarray(inputs["wo"], f32)
    w1 = np.asarray(inputs["w1"], f32)
    w2 = np.asarray(inputs["w2"], f32)
    bq = np.asarray(inputs["bq"], f32)
    bk = np.asarray(inputs["bk"], f32)
    bv = np.asarray(inputs["bv"], f32)
    bo = np.asarray(inputs["bo"], f32)
    b1 = np.asarray(inputs["b1"], f32)
    b2 = np.asarray(inputs["b2"], f32)

    sc = SCH_A / float(np.sqrt(DK))
    # fold LN1 affine into wq/wk/wv; 1/sqrt(DK) and the Schraudolph
    # scale SCH_A into wq/bq; the fp8 range scale WKV_SCALE into wk/wv
    # (undone on the psum).
    wq_f = (a1[:, None] * wq) * sc
    bq_f = (bq + b1n @ wq) * sc
    wk_f = (a1[:, None] * wk) * WKV_SCALE
    bk_f = bk + b1n @ wk
    wv_f = (a1[:, None] * wv) * WKV_SCALE
    bv_f = bv + b1n @ wv
    # fold LN2 affine into w1
    w1_f = a2[:, None] * w1
    b1_f = b1 + b2n @ w1

    # o_sb chunk c holds [head c (rows 0:64) | head c+6 (rows 64:128)]:
    # permute wo's rows to match.
    rperm = np.concatenate(
        [np.r_[c * DK:(c + 1) * DK, (c + 6) * DK:(c + 7) * DK]
         for c in range(6)])
    wo_p = wo[rperm, :]

    fp8 = ml_dtypes.float8_e4m3
    full = {
        "wq": np.ascontiguousarray(wq_f.astype(fp8)),
        "wk": np.ascontiguousarray(wk_f.astype(fp8)),
        "wv": np.ascontiguousarray(wv_f.astype(fp8)),
        "wo": np.ascontiguousarray(wo_p.astype(bf16)),
        "w1": np.ascontiguousarray(w1_f.astype(bf16)),
        "w2": np.ascontiguousarray(w2.astype(bf16)),
        "bvb": np.ascontiguousarray(
            np.broadcast_to(bv_f.astype(bf16), (P, D))),
    }
    for n, v in [("bq", bq_f), ("bk", bk_f), ("bo", bo), ("b2", b2)]:
        full[n] = np.ascontiguousarray(v.reshape(DC, P).T)
    full["b1"] = np.ascontiguousarray(b1_f.reshape(FC, P).T)

    # LN1 precomputed on the host (pure function of the input x)
    mu = x.mean(-1, keepdims=True)
    var = ((x - mu) ** 2).sum(-1, keepdims=True) / (D - 1)
    z = (x - mu) / (np.sqrt(var) + 1e-6)

    in_maps = []
    for c in range(NCORES):
        b, r = divmod(c, 4)
        hT = np.ascontiguousarray(
            np.roll(z[b].T, -TQ * r, axis=1).astype(fp8))
        xres = np.ascontiguousarray(
            x[b, TQ * r:TQ * r + TQ, :].T.astype(bf16))
        mT = np.ascontiguousarray(
            np.roll(mask[0, 0, TQ * r:TQ * r + TQ, :].T.astype(f32),
                    -TQ * r, axis=0).astype(bf16))
        m = dict(full)
        m["hT"] = hT
        m["xres"] = xres
        m["maskT"] = mT
        in_maps.append(m)
    return in_maps


def kernel(**inputs):
    nc = _get_module()
    in_maps = _make_in_maps(inputs)
    from concourse import bass2jax
    results = bass2jax.run_bass_via_pjrt(nc, in_maps, n_cores=NCORES)
    out = np.empty((B, S, D), np.float32)
    for c in range(NCORES):
        b, r = divmod(c, 4)
        out[b, TQ * r:TQ * r + TQ, :] = results[c]["outT"].T
    return out

